# revision 19
# baseline (speedup 1.0000x reference)
"""KGAT forward kernel for 8 Trainium2 NeuronCores (Bass/Tile SPMD).

Strategy (dst-sharded graph parallel), v2:
  - Nodes padded to NP = 8*PPC; core c owns rows [c*PPC, (c+1)*PPC).
  - Node tables T_l [NP, 128] bf16 (64 data cols + 64 pad so each row is a
    256B gather element) replicated in each core's DRAM, image layout:
    flat row of node g = owner*PPC + (loc%128)*TPC + loc//128.
  - Edge phase per layer: per-edge source rows pulled with dma_gather
    (int16 idx, 32768-row windows, 4 SWDGE queues round-robin) directly
    into the matmul operand tile; edge values multiplied in with one
    broadcast tensor_tensor per (group, range); one-hot dst matrices
    generated in batches of KB chunks with a single broadcast is_equal
    tensor_tensor; segment-sum via matmul accumulation in PSUM.
  - Dense phase: X'=[X|1] per tile, PE transpose, bf16 matmul with
    W'=[W;b], leaky-relu on ACT+DVE, l2-norm factors packed into spare
    cols of T3 for scoring.
  - AllGather (collective) replicates each new layer piece (bf16).
  - Scoring: batch shard per core; rows fetched with windowed gathers,
    reordered into batch order with unique-index dma_scatter_add into
    SBUF (parity-split); T0..T2 scoring overlapped with layer compute.
"""
import sys
sys.path.insert(0, '/opt/trn_rl_repo')

import numpy as np
import ml_dtypes

import os
import concourse.bass as bass
import concourse.bacc as bacc
import concourse.tile as tile
from concourse import mybir
from concourse.bass_utils import run_bass_kernel_spmd

BF = ml_dtypes.bfloat16
NCORES = 8
WINDOW = 32768
ACT = mybir.ActivationFunctionType
ALU = mybir.AluOpType
NSWQ = 4       # SWDGE queues
KB = 32        # one-hot chunks per batched generation
GCH = 32       # gather slots per dma_gather instruction


def _wrap16(idx):
    """int16 idx array -> [128, n/16] wrapped+replicated layout."""
    n = len(idx)
    assert n % 16 == 0
    return np.tile(idx.reshape(n // 16, 16).T, (8, 1)).astype(np.int16)


def _img(loc, tpc):
    """local node id -> piece-flat image row."""
    return (loc % 128) * tpc + loc // 128


def build_host_data(inputs):
    """All host-side preprocessing. Returns (meta, in_maps)."""
    users = np.asarray(inputs["users"])
    pos_items = np.asarray(inputs["pos_items"])
    neg_items = np.asarray(inputs["neg_items"])
    rows = np.asarray(inputs["rows"]).astype(np.int64)
    cols = np.asarray(inputs["cols"]).astype(np.int64)
    vals = np.asarray(inputs["edge_vals"]).astype(np.float32)
    ue = np.asarray(inputs["user_embed"]).astype(np.float32)
    ee = np.asarray(inputs["entity_embed"]).astype(np.float32)

    NU, D0 = ue.shape
    NE = ee.shape[0]
    N = NU + NE
    B = users.shape[0]
    BPC = B // NCORES
    B3 = BPC // 128
    SC = 3 * B3

    PPC = -(-N // (NCORES * 128)) * 128
    NP = PPC * NCORES
    TPC = PPC // 128
    NRANGE = -(-NP // WINDOW)

    douts = [inputs[f"W_gc{l}"].shape[1] for l in range(3)]
    dins = [D0, douts[0], douts[1]]

    # --- node -> table flat row (image layout) ---
    def flat_of(g):
        c = g // PPC
        loc = g % PPC
        return c * PPC + _img(loc, TPC)

    # --- ego0 full table [NP, 128] bf16 image layout ---
    allemb = np.concatenate([ue, ee], 0)
    t0 = np.zeros((NP, 128), np.float32)
    t0[flat_of(np.arange(N)), :D0] = allemb
    t0 = t0.astype(BF)

    # --- per-core ego0 piece, SBUF image, bf16 (64-wide packed) ---
    ego0sb = []
    for c in range(NCORES):
        piece = t0[c * PPC:(c + 1) * PPC, :64]      # already image-ordered
        ego0sb.append(np.ascontiguousarray(piece).reshape(128, TPC * 64))

    # --- edge partitioning ---
    core_of = rows // PPC
    dloc = rows - core_of * PPC
    t_of = dloc // 128
    rel_of = (dloc % 128).astype(np.float32)
    srcflat = flat_of(cols)
    r_of = srcflat // WINDOW
    lidx_of = (srcflat - r_of * WINDOW).astype(np.int16)

    # cell (t, r) edge lists per core
    cell_edges = [[[None] * NRANGE for _ in range(TPC)] for _ in range(NCORES)]
    for c in range(NCORES):
        m = core_of == c
        key = t_of[m] * NRANGE + r_of[m]
        order = np.argsort(key, kind="stable")
        eidx = np.nonzero(m)[0][order]
        k = key[order]
        bounds = np.searchsorted(k, np.arange(TPC * NRANGE + 1))
        for t in range(TPC):
            for r in range(NRANGE):
                a, b = bounds[t * NRANGE + r], bounds[t * NRANGE + r + 1]
                cell_edges[c][t][r] = eidx[a:b]

    # uniform chunk counts per (t, r): max over cores
    nchunk = np.zeros((TPC, NRANGE), np.int32)
    for t in range(TPC):
        for r in range(NRANGE):
            mx = max(len(cell_edges[c][t][r]) for c in range(NCORES))
            nchunk[t, r] = -(-mx // 128)

    # groups of tiles
    GT = 7
    groups = [list(range(a, min(a + GT, TPC))) for a in range(0, TPC, GT)]

    # consumption order: q index over (g, t-major, r, j); gather order per (g, r)
    NC = int(nchunk.sum())
    chunk_q = {}
    subk = {}       # (g, r) -> number of chunks in that gather block
    slot_of = {}    # (t, r, j) -> slot in its (g, r) block
    q = 0
    for gi, ts in enumerate(groups):
        for r in range(NRANGE):
            s = 0
            for t in ts:
                for j in range(nchunk[t, r]):
                    slot_of[(t, r, j)] = s
                    s += 1
            subk[(gi, r)] = s
        for t in ts:
            for r in range(NRANGE):
                for j in range(nchunk[t, r]):
                    chunk_q[(t, r, j)] = q
                    q += 1
    assert q == NC

    # per-core edge metadata arrays
    relv = np.full((NCORES, 128, NC), 200.0, np.float32)  # pad lanes: no match
    gidx_parts = {c: [] for c in range(NCORES)}   # per (g, r) int16 arrays
    valslot_parts = {c: [] for c in range(NCORES)}
    gidx_off = {}                                  # (g, r) -> col offset in DRAM [128, ./16]
    slot_base = {}                                 # (g, r) -> global slot offset
    off16 = 0
    slot_off = 0
    for gi, ts in enumerate(groups):
        for r in range(NRANGE):
            sk = subk[(gi, r)]
            if sk == 0:
                continue
            gidx_off[(gi, r)] = off16
            slot_base[(gi, r)] = slot_off
            off16 += sk * 8
            slot_off += sk
            for c in range(NCORES):
                arr = np.zeros(sk * 128, np.int16)
                vsl = np.zeros((128, sk), np.float32)
                for t in ts:
                    for j in range(nchunk[t, r]):
                        s = slot_of[(t, r, j)]
                        e = cell_edges[c][t][r][j * 128:(j + 1) * 128]
                        ne = len(e)
                        arr[s * 128:s * 128 + ne] = lidx_of[e]
                        qq = chunk_q[(t, r, j)]
                        relv[c, :ne, qq] = rel_of[e]
                        vsl[:ne, s] = vals[e]
                gidx_parts[c].append(arr)
                valslot_parts[c].append(vsl)
    gidx = [
        np.concatenate([_wrap16(a) for a in gidx_parts[c]], axis=1)
        for c in range(NCORES)
    ]
    valslot = [
        np.concatenate(valslot_parts[c], axis=1).astype(BF)
        for c in range(NCORES)
    ]
    TOT16 = gidx[0].shape[1]
    TOTSLOT = valslot[0].shape[1]

    # --- weights with bias folded as extra row, bf16 ---
    wmats = {}
    for l in range(3):
        for nm in ("gc", "bi"):
            W = np.asarray(inputs[f"W_{nm}{l}"]).astype(np.float32)
            b = np.asarray(inputs[f"b_{nm}{l}"]).astype(np.float32)
            wmats[f"w_{nm}{l}"] = np.concatenate([W, b.reshape(1, -1)], 0).astype(BF)

    # --- scoring ---
    # staging position for batch b: partition b%128, rank slot 2*(3*(b//128)+which)
    def spos(b, which):
        return (b % 128) + 256 * (3 * (b // 128) + which)

    sg_idx, sg_dst, sg_cnt = [], [], {}
    all_nodes = []
    for c in range(NCORES):
        u = users[c * BPC:(c + 1) * BPC].astype(np.int64)
        p = NU + pos_items[c * BPC:(c + 1) * BPC].astype(np.int64)
        n = NU + neg_items[c * BPC:(c + 1) * BPC].astype(np.int64)
        nodes = np.stack([u, p, n], 1).ravel()       # b-major, (u,p,n)
        which = np.tile(np.array([0, 1, 2]), BPC)
        bb = np.repeat(np.arange(BPC), 3)
        fl = flat_of(nodes)
        all_nodes.append((fl, spos(bb, which)))
    for r in range(NRANGE):
        mx = max(((fl // WINDOW) == r).sum() for fl, _ in all_nodes)
        sg_cnt[r] = int(-(-max(mx, 1) // 128) * 128)
    PADMAX = max(sg_cnt[r] for r in range(NRANGE))
    PEERC = -(-PADMAX // 128) + 1                   # peer dump tile columns
    for c in range(NCORES):
        fl, sp = all_nodes[c]
        iparts, dparts = [], []
        for r in range(NRANGE):
            m = (fl // WINDOW) == r
            cnt = int(m.sum())
            tot = sg_cnt[r]
            gi16 = np.zeros(tot, np.int16)
            gd16 = np.zeros(tot, np.int16)
            gi16[:cnt] = (fl[m] - r * WINDOW).astype(np.int16)
            gd16[:cnt] = sp[m].astype(np.int16)
            padc = 0
            for i in range(cnt, tot):               # unique parity-1 dump slots
                gd16[i] = 128 + (padc % 128) + 256 * (padc // 128)
                padc += 1
            iparts.append(_wrap16(gi16))
            dparts.append(_wrap16(gd16))
        sg_idx.append(np.concatenate(iparts, 1))
        sg_dst.append(np.concatenate(dparts, 1))
    STOT16 = sg_idx[0].shape[1]

    iota = np.tile(np.arange(128, dtype=np.float32), (128, 1)).astype(BF)
    ident = np.eye(128, dtype=np.float32).astype(BF)

    in_maps = []
    for c in range(NCORES):
        m = dict(
            t0=t0,
            ego0sb=ego0sb[c],
            relv=relv[c].astype(BF),
            valslot=valslot[c],
            gidx=gidx[c],
            sgidx=sg_idx[c],
            sgdst=sg_dst[c],
            iota=iota,
            ident=ident,
        )
        m.update(wmats)
        in_maps.append(m)

    meta = dict(
        N=N, NP=NP, PPC=PPC, TPC=TPC, NRANGE=NRANGE, NC=NC,
        BPC=BPC, B3=B3, SC=SC, dins=dins, douts=douts, D0=D0,
        groups=groups, nchunk=nchunk, subk=subk, slot_of=slot_of,
        chunk_q=chunk_q, gidx_off=gidx_off, slot_base=slot_base,
        TOT16=TOT16, TOTSLOT=TOTSLOT,
        sg_cnt=sg_cnt, STOT16=STOT16, B=B, PEERC=PEERC,
    )
    return meta, in_maps


def build_program(meta):
    f32, bf16, i16 = mybir.dt.float32, mybir.dt.bfloat16, mybir.dt.int16
    NP, PPC, TPC = meta["NP"], meta["PPC"], meta["TPC"]
    NRANGE, NC = meta["NRANGE"], meta["NC"]
    B3, SC = meta["B3"], meta["SC"]
    dins, douts = meta["dins"], meta["douts"]
    groups, nchunk = meta["groups"], meta["nchunk"]
    subk, slot_of, chunk_q = meta["subk"], meta["slot_of"], meta["chunk_q"]
    gidx_off, sg_cnt = meta["gidx_off"], meta["sg_cnt"]
    slot_base = meta["slot_base"]
    PEERC = meta["PEERC"]

    nc = bacc.Bacc(num_swdge_queues=NSWQ,
                   dynamic_dma_scratch_size=int(os.environ.get("KGAT_SCRATCH", "32768")))

    t0 = nc.dram_tensor("t0", [NP, 128], bf16, kind="ExternalInput")
    ego0sb_in = nc.dram_tensor("ego0sb", [128, TPC * 64], bf16, kind="ExternalInput")
    relv_in = nc.dram_tensor("relv", [128, NC], bf16, kind="ExternalInput")
    valslot_in = nc.dram_tensor("valslot", [128, meta["TOTSLOT"]], bf16,
                                kind="ExternalInput")
    gidx_in = nc.dram_tensor("gidx", [128, meta["TOT16"]], i16, kind="ExternalInput")
    sgidx_in = nc.dram_tensor("sgidx", [128, meta["STOT16"]], i16, kind="ExternalInput")
    sgdst_in = nc.dram_tensor("sgdst", [128, meta["STOT16"]], i16, kind="ExternalInput")
    iota_in = nc.dram_tensor("iota", [128, 128], bf16, kind="ExternalInput")
    ident_in = nc.dram_tensor("ident", [128, 128], bf16, kind="ExternalInput")
    w_in = {}
    for l in range(3):
        for nm in ("gc", "bi"):
            w_in[f"{nm}{l}"] = nc.dram_tensor(
                f"w_{nm}{l}", [dins[l] + 1, douts[l]], bf16, kind="ExternalInput")
    scores_out = nc.dram_tensor("scores", [128, 2 * B3], f32, kind="ExternalOutput")

    qrr = [0]

    def gather_prep(out_ap, in_ap, idxs_ap, n):
        qn = qrr[0] % NSWQ
        qrr[0] += 1
        nc.gpsimd.dma_gather(
            out_ap=out_ap, in_ap=in_ap, idxs_ap=idxs_ap,
            num_idxs=n, num_idxs_reg=n, elem_size=128,
            single_packet=False, queue_num=qn)

    def scatter_prep(out_ap, in_ap, idxs_ap, n, peer_ap):
        qn = qrr[0] % NSWQ
        qrr[0] += 1
        nc.gpsimd.dma_scatter_add(
            out_ap=out_ap, in_ap=in_ap, idxs_ap=idxs_ap,
            num_idxs=n, num_idxs_reg=n, elem_size=128,
            sbuf_tokens_per_rank=128, parity_reg=0,
            out_ap_other=peer_ap, single_packet=False, queue_num=qn)

    def flush_triggers():
        pass

    with tile.TileContext(nc) as tc:
        with (
            tc.tile_pool(name="const", bufs=1) as cpool,
            tc.tile_pool(name="big", bufs=1) as bigp,
            tc.tile_pool(name="gf", bufs=3) as gfp,
            tc.tile_pool(name="gb", bufs=1) as gbp,
            tc.tile_pool(name="pp", bufs=3) as ppool,
            tc.tile_pool(name="dense", bufs=2) as dpool,
            tc.tile_pool(name="psA", bufs=3, space="PSUM") as psA,
            tc.tile_pool(name="psB", bufs=2, space="PSUM") as psB,
            tc.tile_pool(name="psC", bufs=2, space="PSUM") as psC,
            tc.tile_pool(name="dram", bufs=1, space="DRAM") as dram,
        ):
            # ---- constants ----
            iota_t = cpool.tile([128, 128], bf16, tag="iota")
            nc.sync.dma_start(iota_t[:], iota_in[:])
            ident_t = cpool.tile([128, 128], bf16, tag="ident")
            nc.sync.dma_start(ident_t[:], ident_in[:])
            w_t = {}
            for l in range(3):
                for nm in ("gc", "bi"):
                    w = cpool.tile([dins[l] + 1, douts[l]], bf16, tag=f"w{nm}{l}")
                    nc.sync.dma_start(w[:], w_in[f"{nm}{l}"][:])
                    w_t[f"{nm}{l}"] = w
            relv_t = cpool.tile([128, NC], bf16, tag="relv")
            nc.sync.dma_start(relv_t[:], relv_in[:])
            valslot_t = cpool.tile([128, meta["TOTSLOT"]], bf16, tag="valslot")
            nc.sync.dma_start(valslot_t[:], valslot_in[:])

            # ---- persistent big tiles ----
            ego_sb = bigp.tile([128, TPC * 64], bf16, tag="ego")
            nc.sync.dma_start(ego_sb[:], ego0sb_in[:])
            inv_sb = [bigp.tile([128, TPC], f32, tag=f"inv{l}", name=f"inv{l}")
                      for l in range(3)]

            # ---- DRAM tables / pieces ----
            tables = [t0]
            pieces = []
            for l in range(3):
                T = dram.tile([NP, 128], bf16, tag=f"T{l+1}", name=f"T{l+1}",
                              addr_space="Shared")
                tables.append(T)
                pieces.append(dram.tile([PPC, 128], bf16, tag=f"piece{l+1}",
                                        name=f"piece{l+1}"))

            # ---- scoring staging tiles ----
            stage = []
            for ti in range(4):
                own = bigp.tile([128, SC, 128], bf16, tag=f"stown{ti}")
                peer = bigp.tile([128, SC, 128], bf16, tag=f"stpeer{ti}")
                nc.vector.memset(own[:], 0.0)
                nc.vector.memset(peer[:], 0.0)
                stage.append((own, peer))

            def score_fetch(ti):
                """Gather+reorder scoring rows from tables[ti] into stage[ti]."""
                soff = 0
                own, peer = stage[ti]
                for r in range(NRANGE):
                    tot = sg_cnt[r]
                    gi_t = gfp.tile([128, tot // 16], i16, tag="sgi")
                    nc.sync.dma_start(gi_t[:], sgidx_in[:, soff:soff + tot // 16])
                    gd_t = gfp.tile([128, tot // 16], i16, tag="sgd")
                    nc.sync.dma_start(gd_t[:], sgdst_in[:, soff:soff + tot // 16])
                    soff += tot // 16
                    wsz = min(WINDOW, NP - r * WINDOW)
                    gf = gfp.tile([128, tot // 128, 128], bf16, tag="sgf")
                    gather_prep(gf[:], tables[ti][r * WINDOW:r * WINDOW + wsz],
                                gi_t[:], tot)
                    flush_triggers()
                    scatter_prep(own[:], gf[:], gd_t[:], tot, peer[:])
                    flush_triggers()

            score_fetch(0)

            # ================= layers =================
            for l in range(3):
                din, dout = dins[l], douts[l]
                Tsrc = tables[l]

                for gi, ts in enumerate(groups):
                    gbase = ts[0]
                    Tg = len(ts)
                    # ---- per-group idx preload (one DMA for all ranges) ----
                    rs = [r for r in range(NRANGE) if subk[(gi, r)] > 0]
                    goff0 = gidx_off[(gi, rs[0])]
                    gW = sum(subk[(gi, r)] * 8 for r in rs)
                    idxg = gfp.tile([128, gW], i16, tag="idxg")
                    nc.sync.dma_start(idxg[:], gidx_in[:, goff0:goff0 + gW])
                    # ---- gathers for this group (direct into gb) ----
                    gb_tiles = {}
                    for r in rs:
                        sk = subk[(gi, r)]
                        wsz = min(WINDOW, NP - r * WINDOW)
                        loff = gidx_off[(gi, r)] - goff0
                        gb = gbp.tile([128, sk, 128], bf16, tag=f"gb{r}p{gi % 2}")
                        gb3 = gb[:]
                        for s0 in range(0, sk, GCH):
                            skc = min(GCH, sk - s0)
                            gather_prep(
                                gb3[:, s0:s0 + skc, :],
                                Tsrc[r * WINDOW:r * WINDOW + wsz],
                                idxg[:, loff + s0 * 8:loff + (s0 + skc) * 8],
                                skc * 128)
                        flush_triggers()
                        # scale gathered rows by edge values (broadcast mult)
                        sb = slot_base[(gi, r)]
                        vb = valslot_t[:, sb:sb + sk].rearrange(
                            "p (s o) -> p s o", o=1).broadcast_to((128, sk, din))
                        nc.vector.tensor_tensor(
                            out=gb3[:, :, :din], in0=gb3[:, :, :din], in1=vb,
                            op=ALU.mult)
                        gb_tiles[r] = gb

                    # ---- batched one-hot generation ----
                    qs = [chunk_q[(t, r, j)] for t in ts for r in range(NRANGE)
                          for j in range(int(nchunk[t, r]))]
                    if not qs:
                        continue
                    qg0, qg1 = min(qs), max(qs) + 1
                    nq = qg1 - qg0
                    P_tiles = {}
                    for b0 in range(0, nq, KB):
                        nb = min(KB, nq - b0)
                        P = ppool.tile([128, KB, 128], bf16, tag="P")
                        rb = relv_t[:, qg0 + b0:qg0 + b0 + nb].rearrange(
                            "p (q o) -> p q o", o=1).broadcast_to((128, nb, 128))
                        ib = iota_t[:].rearrange(
                            "p (o i) -> p o i", o=1).broadcast_to((128, nb, 128))
                        nc.vector.tensor_tensor(
                            out=P[:, :nb, :], in0=ib, in1=rb, op=ALU.is_equal)
                        P_tiles[b0] = P

                    # ---- one-hot matmul accumulation per tile + psum copy ----
                    sideg = dpool.tile([128, Tg, 64], bf16, tag="sideg")
                    for t in ts:
                        tot = int(nchunk[t].sum())
                        if tot == 0:
                            nc.vector.memset(sideg[:, t - gbase, :din], 0.0)
                            continue
                        ps = psA.tile([128, 64], f32, tag="ps")
                        done = 0
                        for r in range(NRANGE):
                            for j in range(int(nchunk[t, r])):
                                qq = chunk_q[(t, r, j)]
                                s = slot_of[(t, r, j)]
                                b0 = ((qq - qg0) // KB) * KB
                                qo = qq - qg0 - b0
                                nc.tensor.matmul(
                                    ps[:, :din],
                                    lhsT=P_tiles[b0][:, qo, :],
                                    rhs=gb_tiles[r][:][:, s, :din],
                                    start=(done == 0), stop=(done == tot - 1),
                                )
                                done += 1
                        nc.scalar.copy(out=sideg[:, t - gbase, :din],
                                       in_=ps[:, :din])

                    # ---- dense phase for this group ----
                    sl_ego = ego_sb[:, gbase * 64:(gbase + Tg) * 64].rearrange(
                        "p (t d) -> p t d", d=64)[:, :, :din]
                    plus = dpool.tile([128, Tg, din + 1], bf16, tag="plus")
                    nc.vector.tensor_tensor(out=plus[:, :, :din],
                                            in0=sideg[:, :, :din], in1=sl_ego,
                                            op=ALU.add)
                    nc.vector.memset(plus[:, :, din], 1.0)
                    times = dpool.tile([128, Tg, din + 1], bf16, tag="times")
                    nc.vector.tensor_tensor(out=times[:, :, :din],
                                            in0=sideg[:, :, :din], in1=sl_ego,
                                            op=ALU.mult)
                    nc.vector.memset(times[:, :, din], 1.0)
                    bo = {}
                    for bname, src, wkey in (("p", plus, f"gc{l}"),
                                             ("b", times, f"bi{l}")):
                        out_b = dpool.tile([128, Tg * dout], f32, tag=f"bo{bname}")
                        for ti in range(Tg):
                            tp = psB.tile([din + 1, 128], bf16, tag="tp")
                            nc.tensor.transpose(tp[:], src[:, ti, :], ident_t[:])
                            xt = ppool.tile([din + 1, 128], bf16, tag="xt")
                            nc.scalar.copy(out=xt[:], in_=tp[:])
                            mo = psC.tile([128, dout], f32, tag="mo")
                            nc.tensor.matmul(mo[:], lhsT=xt[:], rhs=w_t[wkey][:],
                                             start=True, stop=True)
                            ob = out_b[:, ti * dout:(ti + 1) * dout]
                            nc.scalar.activation(ob, mo[:], ACT.Copy, scale=0.01)
                            nc.vector.tensor_tensor(out=ob, in0=ob, in1=mo[:],
                                                    op=ALU.max)
                        bo[bname] = out_b
                    out_g = dpool.tile([128, Tg * 64], f32, tag="outg")
                    nc.vector.memset(out_g[:], 0.0)
                    og3 = out_g[:].rearrange("p (t d) -> p t d", d=64)[:, :, :dout]
                    nc.vector.tensor_tensor(out=og3, in0=bo["p"][:], in1=bo["b"][:],
                                            op=ALU.add)
                    # l2 norm factors
                    sq = dpool.tile([128, Tg * dout], f32, tag="sq")
                    nc.vector.tensor_tensor(out=sq[:], in0=og3, in1=og3, op=ALU.mult)
                    ssum = dpool.tile([128, Tg], f32, tag="ssum")
                    nc.vector.reduce_sum(
                        out=ssum[:], in_=sq[:].rearrange("p (t d) -> p t d", d=dout),
                        axis=mybir.AxisListType.X)
                    nrm = dpool.tile([128, Tg], f32, tag="nrm")
                    nc.scalar.activation(nrm[:], ssum[:], ACT.Sqrt)
                    nc.vector.tensor_scalar_max(out=nrm[:], in0=nrm[:], scalar1=1e-12)
                    nc.vector.reciprocal(inv_sb[l][:, gbase:gbase + Tg], nrm[:])
                    if l == 2:
                        og64 = out_g[:].rearrange("p (t d) -> p t d", d=64)
                        for i in range(3):
                            nc.vector.tensor_copy(
                                out=og64[:, :, 16 + i],
                                in_=inv_sb[i][:, gbase:gbase + Tg])
                    # update ego (bf16) and store piece (padded bf16)
                    nc.vector.tensor_copy(
                        out=ego_sb[:, gbase * 64:(gbase + Tg) * 64], in_=out_g[:])
                    staged = dpool.tile([128, Tg, 128], bf16, tag="staged")
                    nc.vector.memset(staged[:, :, 64:], 0.0)
                    nc.vector.tensor_copy(
                        out=staged[:, :, :64],
                        in_=out_g[:].rearrange("p (t d) -> p t d", d=64))
                    dstp = pieces[l][:].rearrange("(p t) d -> p (t d)", p=128)
                    nc.sync.dma_start(
                        dstp[:, gbase * 128:(gbase + Tg) * 128],
                        staged[:].rearrange("p t d -> p (t d)"))

                nc.gpsimd.collective_compute(
                    "AllGather", ALU.bypass,
                    replica_groups=[list(range(NCORES))],
                    ins=[pieces[l].opt()], outs=[tables[l + 1].opt()],
                )
                score_fetch(l + 1)

            # ================= scoring dots =================
            dls = [meta["D0"]] + douts
            acc = {}
            for which, o1 in (("pos", 128), ("neg", 256)):
                total = dpool.tile([128, B3], f32, tag=f"tot{which}")
                for ti in range(4):
                    own = stage[ti][0]
                    dl = dls[ti]
                    flat = own[:].rearrange("p c d -> p (c d)").rearrange(
                        "p (j x) -> p j x", x=384)
                    u = flat[:, :, 0:dl]
                    v = flat[:, :, o1:o1 + dl]
                    prod = dpool.tile([128, B3 * dl], f32, tag="prod")
                    nc.vector.tensor_tensor(out=prod[:], in0=u, in1=v, op=ALU.mult)
                    d = dpool.tile([128, B3], f32, tag=f"dot{ti}{which}")
                    nc.vector.reduce_sum(
                        out=d[:], in_=prod[:].rearrange("p (j d) -> p j d", d=dl),
                        axis=mybir.AxisListType.X)
                    acc[(ti, which)] = d
                own3 = stage[3][0]
                flat3 = own3[:].rearrange("p c d -> p (c d)").rearrange(
                    "p (j x) -> p j x", x=384)
                for ti in range(1, 4):
                    col = 16 + ti - 1
                    iu = flat3[:, :, col]
                    iv = flat3[:, :, o1 + col]
                    d = acc[(ti, which)]
                    nc.vector.tensor_tensor(out=d[:], in0=d[:], in1=iu, op=ALU.mult)
                    nc.vector.tensor_tensor(out=d[:], in0=d[:], in1=iv, op=ALU.mult)
                nc.vector.tensor_tensor(out=total[:], in0=acc[(0, which)][:],
                                        in1=acc[(1, which)][:], op=ALU.add)
                nc.vector.tensor_tensor(out=total[:], in0=total[:],
                                        in1=acc[(2, which)][:], op=ALU.add)
                nc.vector.tensor_tensor(out=total[:], in0=total[:],
                                        in1=acc[(3, which)][:], op=ALU.add)
                acc[which] = total
            outt = dpool.tile([128, 2 * B3], f32, tag="outt")
            nc.vector.tensor_copy(out=outt[:, :B3], in_=acc["pos"][:])
            nc.vector.tensor_copy(out=outt[:, B3:], in_=acc["neg"][:])
            nc.sync.dma_start(scores_out[:], outt[:])

    nc.compile()
    return nc


def kernel(**inputs):
    meta, in_maps = build_host_data(inputs)
    nc = build_program(meta)
    trace = os.environ.get("KGAT_TRACE", "0") == "1"
    rr = run_bass_kernel_spmd(nc, in_maps, list(range(NCORES)), trace=trace)
    if trace and rr.exec_time_ns is not None:
        print(f"HW exec time: {rr.exec_time_ns} ns")
    if trace and rr.profile_json is not None:
        import json
        with open("/tmp/kgat_profile.json", "w") as f:
            json.dump(rr.profile_json, f)
    res = rr.results
    B3, BPC, B = meta["B3"], meta["BPC"], meta["B"]
    out = np.zeros((B, 2), np.float32)
    for c in range(NCORES):
        sc = res[c]["scores"]                       # [128, 2*B3]
        pos = sc[:, :B3]                            # [128, B3] (partition, jj)
        neg = sc[:, B3:]
        b = np.arange(BPC)
        out[c * BPC + b, 0] = pos[b % 128, b // 128]
        out[c * BPC + b, 1] = neg[b % 128, b // 128]
    return out


# revision 22
# speedup vs baseline: 12.0180x; 12.0180x over previous
"""KGAT forward kernel for 8 Trainium2 NeuronCores (Bass/Tile SPMD).

Strategy (dst-sharded graph parallel), v2:
  - Nodes padded to NP = 8*PPC; core c owns rows [c*PPC, (c+1)*PPC).
  - Node tables T_l [NP, 128] bf16 (64 data cols + 64 pad so each row is a
    256B gather element) replicated in each core's DRAM, image layout:
    flat row of node g = owner*PPC + (loc%128)*TPC + loc//128.
  - Edge phase per layer: per-edge source rows pulled with dma_gather
    (int16 idx, 32768-row windows, 4 SWDGE queues round-robin) directly
    into the matmul operand tile; edge values multiplied in with one
    broadcast tensor_tensor per (group, range); one-hot dst matrices
    generated in batches of KB chunks with a single broadcast is_equal
    tensor_tensor; segment-sum via matmul accumulation in PSUM.
  - Dense phase: X'=[X|1] per tile, PE transpose, bf16 matmul with
    W'=[W;b], leaky-relu on ACT+DVE, l2-norm factors packed into spare
    cols of T3 for scoring.
  - AllGather (collective) replicates each new layer piece (bf16).
  - Scoring: batch shard per core; rows fetched with windowed gathers,
    reordered into batch order with unique-index dma_scatter_add into
    SBUF (parity-split); T0..T2 scoring overlapped with layer compute.
"""
import sys
sys.path.insert(0, '/opt/trn_rl_repo')

import numpy as np
import ml_dtypes

import os
import concourse.bass as bass
import concourse.bacc as bacc
import concourse.tile as tile
from concourse import mybir
from concourse.bass_utils import run_bass_kernel_spmd

BF = ml_dtypes.bfloat16
NCORES = 8
WINDOW = 32768
ACT = mybir.ActivationFunctionType
ALU = mybir.AluOpType
NSWQ = 4       # SWDGE queues
KB = 32        # one-hot chunks per batched generation
GCH = 32       # gather slots per dma_gather instruction
SP = os.environ.get("KGAT_SP", "0") == "1"   # single_packet for edge gathers


def _wrap16(idx):
    """int16 idx array -> [128, n/16] wrapped+replicated layout."""
    n = len(idx)
    assert n % 16 == 0
    return np.tile(idx.reshape(n // 16, 16).T, (8, 1)).astype(np.int16)


def _img(loc, tpc):
    """local node id -> piece-flat image row."""
    return (loc % 128) * tpc + loc // 128


def build_host_data(inputs):
    """All host-side preprocessing. Returns (meta, in_maps)."""
    users = np.asarray(inputs["users"])
    pos_items = np.asarray(inputs["pos_items"])
    neg_items = np.asarray(inputs["neg_items"])
    rows = np.asarray(inputs["rows"]).astype(np.int64)
    cols = np.asarray(inputs["cols"]).astype(np.int64)
    vals = np.asarray(inputs["edge_vals"]).astype(np.float32)
    ue = np.asarray(inputs["user_embed"]).astype(np.float32)
    ee = np.asarray(inputs["entity_embed"]).astype(np.float32)

    NU, D0 = ue.shape
    NE = ee.shape[0]
    N = NU + NE
    B = users.shape[0]
    BPC = B // NCORES
    B3 = BPC // 128
    SC = 3 * B3

    PPC = -(-N // (NCORES * 128)) * 128
    NP = PPC * NCORES
    TPC = PPC // 128
    NRANGE = -(-NP // WINDOW)

    douts = [inputs[f"W_gc{l}"].shape[1] for l in range(3)]
    dins = [D0, douts[0], douts[1]]

    # --- node -> table flat row (image layout) ---
    def flat_of(g):
        c = g // PPC
        loc = g % PPC
        return c * PPC + _img(loc, TPC)

    # --- ego0 full table [NP, 128] bf16 image layout ---
    allemb = np.concatenate([ue, ee], 0)
    t0 = np.zeros((NP, 128), np.float32)
    t0[flat_of(np.arange(N)), :D0] = allemb
    t0 = t0.astype(BF)

    # --- per-core ego0 piece, SBUF image, bf16 (64-wide packed) ---
    ego0sb = []
    for c in range(NCORES):
        piece = t0[c * PPC:(c + 1) * PPC, :64]      # already image-ordered
        ego0sb.append(np.ascontiguousarray(piece).reshape(128, TPC * 64))

    # --- edge partitioning ---
    core_of = rows // PPC
    dloc = rows - core_of * PPC
    t_of = dloc // 128
    rel_of = (dloc % 128).astype(np.float32)
    srcflat = flat_of(cols)
    r_of = srcflat // WINDOW
    lidx_of = (srcflat - r_of * WINDOW).astype(np.int16)

    # cell (t, r) edge lists per core
    cell_edges = [[[None] * NRANGE for _ in range(TPC)] for _ in range(NCORES)]
    for c in range(NCORES):
        m = core_of == c
        key = t_of[m] * NRANGE + r_of[m]
        order = np.argsort(key, kind="stable")
        eidx = np.nonzero(m)[0][order]
        k = key[order]
        bounds = np.searchsorted(k, np.arange(TPC * NRANGE + 1))
        for t in range(TPC):
            for r in range(NRANGE):
                a, b = bounds[t * NRANGE + r], bounds[t * NRANGE + r + 1]
                cell_edges[c][t][r] = eidx[a:b]

    # uniform chunk counts per (t, r): max over cores
    nchunk = np.zeros((TPC, NRANGE), np.int32)
    for t in range(TPC):
        for r in range(NRANGE):
            mx = max(len(cell_edges[c][t][r]) for c in range(NCORES))
            nchunk[t, r] = -(-mx // 128)

    # groups of tiles
    GT = 7
    groups = [list(range(a, min(a + GT, TPC))) for a in range(0, TPC, GT)]

    # consumption order: q index over (g, t-major, r, j); gather order per (g, r)
    NC = int(nchunk.sum())
    chunk_q = {}
    subk = {}       # (g, r) -> number of chunks in that gather block
    slot_of = {}    # (t, r, j) -> slot in its (g, r) block
    q = 0
    for gi, ts in enumerate(groups):
        for r in range(NRANGE):
            s = 0
            for t in ts:
                for j in range(nchunk[t, r]):
                    slot_of[(t, r, j)] = s
                    s += 1
            subk[(gi, r)] = s
        for t in ts:
            for r in range(NRANGE):
                for j in range(nchunk[t, r]):
                    chunk_q[(t, r, j)] = q
                    q += 1
    assert q == NC

    # per-core edge metadata arrays
    relv = np.full((NCORES, 128, NC), 200.0, np.float32)  # pad lanes: no match
    gidx_parts = {c: [] for c in range(NCORES)}   # per (g, r) int16 arrays
    valslot_parts = {c: [] for c in range(NCORES)}
    gidx_off = {}                                  # (g, r) -> col offset in DRAM [128, ./16]
    slot_base = {}                                 # (g, r) -> global slot offset
    off16 = 0
    slot_off = 0
    for gi, ts in enumerate(groups):
        for r in range(NRANGE):
            sk = subk[(gi, r)]
            if sk == 0:
                continue
            gidx_off[(gi, r)] = off16
            slot_base[(gi, r)] = slot_off
            off16 += sk * 8
            slot_off += sk
            for c in range(NCORES):
                arr = np.zeros(sk * 128, np.int16)
                vsl = np.zeros((128, sk), np.float32)
                for t in ts:
                    for j in range(nchunk[t, r]):
                        s = slot_of[(t, r, j)]
                        e = cell_edges[c][t][r][j * 128:(j + 1) * 128]
                        ne = len(e)
                        arr[s * 128:s * 128 + ne] = lidx_of[e]
                        qq = chunk_q[(t, r, j)]
                        relv[c, :ne, qq] = rel_of[e]
                        vsl[:ne, s] = vals[e]
                gidx_parts[c].append(arr)
                valslot_parts[c].append(vsl)
    gidx = [
        np.concatenate([_wrap16(a) for a in gidx_parts[c]], axis=1)
        for c in range(NCORES)
    ]
    valslot = [
        np.concatenate(valslot_parts[c], axis=1).astype(BF)
        for c in range(NCORES)
    ]
    TOT16 = gidx[0].shape[1]
    TOTSLOT = valslot[0].shape[1]

    # --- weights with bias folded as extra row, bf16 ---
    wmats = {}
    for l in range(3):
        for nm in ("gc", "bi"):
            W = np.asarray(inputs[f"W_{nm}{l}"]).astype(np.float32)
            b = np.asarray(inputs[f"b_{nm}{l}"]).astype(np.float32)
            wmats[f"w_{nm}{l}"] = np.concatenate([W, b.reshape(1, -1)], 0).astype(BF)

    # --- scoring ---
    # staging position for batch b: partition b%128, rank slot 2*(3*(b//128)+which)
    def spos(b, which):
        return (b % 128) + 256 * (3 * (b // 128) + which)

    sg_idx, sg_dst, sg_cnt = [], [], {}
    all_nodes = []
    for c in range(NCORES):
        u = users[c * BPC:(c + 1) * BPC].astype(np.int64)
        p = NU + pos_items[c * BPC:(c + 1) * BPC].astype(np.int64)
        n = NU + neg_items[c * BPC:(c + 1) * BPC].astype(np.int64)
        nodes = np.stack([u, p, n], 1).ravel()       # b-major, (u,p,n)
        which = np.tile(np.array([0, 1, 2]), BPC)
        bb = np.repeat(np.arange(BPC), 3)
        fl = flat_of(nodes)
        all_nodes.append((fl, spos(bb, which)))
    for r in range(NRANGE):
        mx = max(((fl // WINDOW) == r).sum() for fl, _ in all_nodes)
        sg_cnt[r] = int(-(-max(mx, 1) // 128) * 128)
    PADMAX = max(sg_cnt[r] for r in range(NRANGE))
    PEERC = -(-PADMAX // 128) + 1                   # peer dump tile columns
    for c in range(NCORES):
        fl, sp = all_nodes[c]
        iparts, dparts = [], []
        for r in range(NRANGE):
            m = (fl // WINDOW) == r
            cnt = int(m.sum())
            tot = sg_cnt[r]
            gi16 = np.zeros(tot, np.int16)
            gd16 = np.zeros(tot, np.int16)
            gi16[:cnt] = (fl[m] - r * WINDOW).astype(np.int16)
            gd16[:cnt] = sp[m].astype(np.int16)
            padc = 0
            for i in range(cnt, tot):               # unique parity-1 dump slots
                gd16[i] = 128 + (padc % 128) + 256 * (padc // 128)
                padc += 1
            iparts.append(_wrap16(gi16))
            dparts.append(_wrap16(gd16))
        sg_idx.append(np.concatenate(iparts, 1))
        sg_dst.append(np.concatenate(dparts, 1))
    STOT16 = sg_idx[0].shape[1]

    iota = np.tile(np.arange(128, dtype=np.float32), (128, 1)).astype(BF)
    ident = np.eye(128, dtype=np.float32).astype(BF)

    in_maps = []
    for c in range(NCORES):
        m = dict(
            t0=t0,
            ego0sb=ego0sb[c],
            relv=relv[c].astype(BF),
            valslot=valslot[c],
            gidx=gidx[c],
            sgidx=sg_idx[c],
            sgdst=sg_dst[c],
            iota=iota,
            ident=ident,
        )
        m.update(wmats)
        in_maps.append(m)

    meta = dict(
        N=N, NP=NP, PPC=PPC, TPC=TPC, NRANGE=NRANGE, NC=NC,
        BPC=BPC, B3=B3, SC=SC, dins=dins, douts=douts, D0=D0,
        groups=groups, nchunk=nchunk, subk=subk, slot_of=slot_of,
        chunk_q=chunk_q, gidx_off=gidx_off, slot_base=slot_base,
        TOT16=TOT16, TOTSLOT=TOTSLOT,
        sg_cnt=sg_cnt, STOT16=STOT16, B=B, PEERC=PEERC,
    )
    return meta, in_maps


def build_program(meta):
    f32, bf16, i16 = mybir.dt.float32, mybir.dt.bfloat16, mybir.dt.int16
    NP, PPC, TPC = meta["NP"], meta["PPC"], meta["TPC"]
    NRANGE, NC = meta["NRANGE"], meta["NC"]
    B3, SC = meta["B3"], meta["SC"]
    dins, douts = meta["dins"], meta["douts"]
    groups, nchunk = meta["groups"], meta["nchunk"]
    subk, slot_of, chunk_q = meta["subk"], meta["slot_of"], meta["chunk_q"]
    gidx_off, sg_cnt = meta["gidx_off"], meta["sg_cnt"]
    slot_base = meta["slot_base"]
    PEERC = meta["PEERC"]

    nc = bacc.Bacc(num_swdge_queues=NSWQ,
                   dynamic_dma_scratch_size=int(os.environ.get("KGAT_SCRATCH", "32768")))

    t0 = nc.dram_tensor("t0", [NP, 128], bf16, kind="ExternalInput")
    ego0sb_in = nc.dram_tensor("ego0sb", [128, TPC * 64], bf16, kind="ExternalInput")
    relv_in = nc.dram_tensor("relv", [128, NC], bf16, kind="ExternalInput")
    valslot_in = nc.dram_tensor("valslot", [128, meta["TOTSLOT"]], bf16,
                                kind="ExternalInput")
    gidx_in = nc.dram_tensor("gidx", [128, meta["TOT16"]], i16, kind="ExternalInput")
    sgidx_in = nc.dram_tensor("sgidx", [128, meta["STOT16"]], i16, kind="ExternalInput")
    sgdst_in = nc.dram_tensor("sgdst", [128, meta["STOT16"]], i16, kind="ExternalInput")
    iota_in = nc.dram_tensor("iota", [128, 128], bf16, kind="ExternalInput")
    ident_in = nc.dram_tensor("ident", [128, 128], bf16, kind="ExternalInput")
    w_in = {}
    for l in range(3):
        for nm in ("gc", "bi"):
            w_in[f"{nm}{l}"] = nc.dram_tensor(
                f"w_{nm}{l}", [dins[l] + 1, douts[l]], bf16, kind="ExternalInput")
    scores_out = nc.dram_tensor("scores", [128, 2 * B3], f32, kind="ExternalOutput")

    qrr = [0]

    def gather_prep(out_ap, in_ap, idxs_ap, n):
        qn = qrr[0] % NSWQ
        qrr[0] += 1
        nc.gpsimd.dma_gather(
            out_ap=out_ap, in_ap=in_ap, idxs_ap=idxs_ap,
            num_idxs=n, num_idxs_reg=n, elem_size=128,
            single_packet=SP, queue_num=qn)

    def scatter_prep(out_ap, in_ap, idxs_ap, n, peer_ap):
        qn = qrr[0] % NSWQ
        qrr[0] += 1
        nc.gpsimd.dma_scatter_add(
            out_ap=out_ap, in_ap=in_ap, idxs_ap=idxs_ap,
            num_idxs=n, num_idxs_reg=n, elem_size=128,
            sbuf_tokens_per_rank=128, parity_reg=0,
            out_ap_other=peer_ap, single_packet=False, queue_num=qn)

    def flush_triggers():
        pass

    with tile.TileContext(nc) as tc:
        with (
            tc.tile_pool(name="const", bufs=1) as cpool,
            tc.tile_pool(name="big", bufs=1) as bigp,
            tc.tile_pool(name="gf", bufs=3) as gfp,
            tc.tile_pool(name="gb", bufs=1) as gbp,
            tc.tile_pool(name="pp", bufs=4) as ppool,
            tc.tile_pool(name="dense", bufs=2) as dpool,
            tc.tile_pool(name="psA", bufs=3, space="PSUM") as psA,
            tc.tile_pool(name="psB", bufs=2, space="PSUM") as psB,
            tc.tile_pool(name="psC", bufs=2, space="PSUM") as psC,
            tc.tile_pool(name="dram", bufs=1, space="DRAM") as dram,
        ):
            # ---- constants ----
            iota_t = cpool.tile([128, 128], bf16, tag="iota")
            nc.sync.dma_start(iota_t[:], iota_in[:])
            ident_t = cpool.tile([128, 128], bf16, tag="ident")
            nc.sync.dma_start(ident_t[:], ident_in[:])
            w_t = {}
            for l in range(3):
                for nm in ("gc", "bi"):
                    w = cpool.tile([dins[l] + 1, douts[l]], bf16, tag=f"w{nm}{l}")
                    nc.sync.dma_start(w[:], w_in[f"{nm}{l}"][:])
                    w_t[f"{nm}{l}"] = w
            relv_t = cpool.tile([128, NC], bf16, tag="relv")
            nc.sync.dma_start(relv_t[:], relv_in[:])
            valslot_t = cpool.tile([128, meta["TOTSLOT"]], bf16, tag="valslot")
            nc.sync.dma_start(valslot_t[:], valslot_in[:])

            # ---- persistent big tiles ----
            ego_sb = bigp.tile([128, TPC * 64], bf16, tag="ego")
            nc.sync.dma_start(ego_sb[:], ego0sb_in[:])
            inv_sb = [bigp.tile([128, TPC], f32, tag=f"inv{l}", name=f"inv{l}")
                      for l in range(3)]

            # ---- DRAM tables / pieces ----
            tables = [t0]
            pieces = []
            for l in range(3):
                T = dram.tile([NP, 128], bf16, tag=f"T{l+1}", name=f"T{l+1}",
                              addr_space="Shared")
                tables.append(T)
                pieces.append(dram.tile([PPC, 128], bf16, tag=f"piece{l+1}",
                                        name=f"piece{l+1}"))

            # ---- scoring staging tiles ----
            stage = []
            for ti in range(4):
                own = bigp.tile([128, SC, 128], bf16, tag=f"stown{ti}")
                peer = bigp.tile([128, SC, 128], bf16, tag=f"stpeer{ti}")
                nc.vector.memset(own[:], 0.0)
                nc.vector.memset(peer[:], 0.0)
                stage.append((own, peer))

            def score_fetch(ti):
                """Gather+reorder scoring rows from tables[ti] into stage[ti]."""
                soff = 0
                own, peer = stage[ti]
                for r in range(NRANGE):
                    tot = sg_cnt[r]
                    gi_t = gfp.tile([128, tot // 16], i16, tag="sgi")
                    nc.sync.dma_start(gi_t[:], sgidx_in[:, soff:soff + tot // 16])
                    gd_t = gfp.tile([128, tot // 16], i16, tag="sgd")
                    nc.sync.dma_start(gd_t[:], sgdst_in[:, soff:soff + tot // 16])
                    soff += tot // 16
                    wsz = min(WINDOW, NP - r * WINDOW)
                    gf = gfp.tile([128, tot // 128, 128], bf16, tag="sgf")
                    gather_prep(gf[:], tables[ti][r * WINDOW:r * WINDOW + wsz],
                                gi_t[:], tot)
                    flush_triggers()
                    scatter_prep(own[:], gf[:], gd_t[:], tot, peer[:])
                    flush_triggers()

            score_fetch(0)

            # ================= layers =================
            for l in range(3):
                din, dout = dins[l], douts[l]
                Tsrc = tables[l]

                for gi, ts in enumerate(groups):
                    gbase = ts[0]
                    Tg = len(ts)
                    # ---- per-group idx preload (one DMA for all ranges) ----
                    rs = [r for r in range(NRANGE) if subk[(gi, r)] > 0]
                    goff0 = gidx_off[(gi, rs[0])]
                    gW = sum(subk[(gi, r)] * 8 for r in rs)
                    idxg = gfp.tile([128, gW], i16, tag="idxg")
                    nc.sync.dma_start(idxg[:], gidx_in[:, goff0:goff0 + gW])
                    # ---- gathers for this group (direct into gb) ----
                    gb_tiles = {}
                    for r in rs:
                        sk = subk[(gi, r)]
                        wsz = min(WINDOW, NP - r * WINDOW)
                        loff = gidx_off[(gi, r)] - goff0
                        gb = gbp.tile([128, sk, 128], bf16, tag=f"gb{r}p{gi % 2}")
                        gb3 = gb[:]
                        for s0 in range(0, sk, GCH):
                            skc = min(GCH, sk - s0)
                            gather_prep(
                                gb3[:, s0:s0 + skc, :],
                                Tsrc[r * WINDOW:r * WINDOW + wsz],
                                idxg[:, loff + s0 * 8:loff + (s0 + skc) * 8],
                                skc * 128)
                        flush_triggers()
                        # scale gathered rows by edge values (broadcast mult)
                        sb = slot_base[(gi, r)]
                        vb = valslot_t[:, sb:sb + sk].rearrange(
                            "p (s o) -> p s o", o=1).broadcast_to((128, sk, din))
                        nc.vector.tensor_tensor(
                            out=gb3[:, :, :din], in0=gb3[:, :, :din], in1=vb,
                            op=ALU.mult)
                        gb_tiles[r] = gb

                    # ---- batched one-hot generation ----
                    qs = [chunk_q[(t, r, j)] for t in ts for r in range(NRANGE)
                          for j in range(int(nchunk[t, r]))]
                    if not qs:
                        continue
                    qg0, qg1 = min(qs), max(qs) + 1
                    nq = qg1 - qg0
                    P_tiles = {}
                    for b0 in range(0, nq, KB):
                        nb = min(KB, nq - b0)
                        P = ppool.tile([128, KB, 128], bf16, tag="P")
                        rb = relv_t[:, qg0 + b0:qg0 + b0 + nb].rearrange(
                            "p (q o) -> p q o", o=1).broadcast_to((128, nb, 128))
                        ib = iota_t[:].rearrange(
                            "p (o i) -> p o i", o=1).broadcast_to((128, nb, 128))
                        nc.vector.tensor_tensor(
                            out=P[:, :nb, :], in0=ib, in1=rb, op=ALU.is_equal)
                        P_tiles[b0] = P

                    # ---- one-hot matmul accumulation per tile + psum copy ----
                    sideg = dpool.tile([128, Tg, 64], bf16, tag="sideg")
                    for t in ts:
                        tot = int(nchunk[t].sum())
                        if tot == 0:
                            nc.vector.memset(sideg[:, t - gbase, :din], 0.0)
                            continue
                        ps = psA.tile([128, 64], f32, tag="ps")
                        done = 0
                        for r in range(NRANGE):
                            for j in range(int(nchunk[t, r])):
                                qq = chunk_q[(t, r, j)]
                                s = slot_of[(t, r, j)]
                                b0 = ((qq - qg0) // KB) * KB
                                qo = qq - qg0 - b0
                                nc.tensor.matmul(
                                    ps[:, :din],
                                    lhsT=P_tiles[b0][:, qo, :],
                                    rhs=gb_tiles[r][:][:, s, :din],
                                    start=(done == 0), stop=(done == tot - 1),
                                )
                                done += 1
                        nc.scalar.copy(out=sideg[:, t - gbase, :din],
                                       in_=ps[:, :din])

                    # ---- dense phase for this group ----
                    sl_ego = ego_sb[:, gbase * 64:(gbase + Tg) * 64].rearrange(
                        "p (t d) -> p t d", d=64)[:, :, :din]
                    plus = dpool.tile([128, Tg, din + 1], bf16, tag="plus")
                    nc.vector.tensor_tensor(out=plus[:, :, :din],
                                            in0=sideg[:, :, :din], in1=sl_ego,
                                            op=ALU.add)
                    nc.vector.memset(plus[:, :, din], 1.0)
                    times = dpool.tile([128, Tg, din + 1], bf16, tag="times")
                    nc.vector.tensor_tensor(out=times[:, :, :din],
                                            in0=sideg[:, :, :din], in1=sl_ego,
                                            op=ALU.mult)
                    nc.vector.memset(times[:, :, din], 1.0)
                    bo = {}
                    for bname, src, wkey in (("p", plus, f"gc{l}"),
                                             ("b", times, f"bi{l}")):
                        out_b = dpool.tile([128, Tg * dout], f32, tag=f"bo{bname}")
                        for ti in range(Tg):
                            tp = psB.tile([din + 1, 128], bf16, tag="tp")
                            nc.tensor.transpose(tp[:], src[:, ti, :], ident_t[:])
                            xt = ppool.tile([din + 1, 128], bf16, tag="xt")
                            nc.scalar.copy(out=xt[:], in_=tp[:])
                            mo = psC.tile([128, dout], f32, tag="mo")
                            nc.tensor.matmul(mo[:], lhsT=xt[:], rhs=w_t[wkey][:],
                                             start=True, stop=True)
                            ob = out_b[:, ti * dout:(ti + 1) * dout]
                            nc.scalar.activation(ob, mo[:], ACT.Copy, scale=0.01)
                            nc.vector.tensor_tensor(out=ob, in0=ob, in1=mo[:],
                                                    op=ALU.max)
                        bo[bname] = out_b
                    out_g = dpool.tile([128, Tg * 64], f32, tag="outg")
                    nc.vector.memset(out_g[:], 0.0)
                    og3 = out_g[:].rearrange("p (t d) -> p t d", d=64)[:, :, :dout]
                    nc.vector.tensor_tensor(out=og3, in0=bo["p"][:], in1=bo["b"][:],
                                            op=ALU.add)
                    # l2 norm factors
                    sq = dpool.tile([128, Tg * dout], f32, tag="sq")
                    nc.vector.tensor_tensor(out=sq[:], in0=og3, in1=og3, op=ALU.mult)
                    ssum = dpool.tile([128, Tg], f32, tag="ssum")
                    nc.vector.reduce_sum(
                        out=ssum[:], in_=sq[:].rearrange("p (t d) -> p t d", d=dout),
                        axis=mybir.AxisListType.X)
                    nrm = dpool.tile([128, Tg], f32, tag="nrm")
                    nc.scalar.activation(nrm[:], ssum[:], ACT.Sqrt)
                    nc.vector.tensor_scalar_max(out=nrm[:], in0=nrm[:], scalar1=1e-12)
                    nc.vector.reciprocal(inv_sb[l][:, gbase:gbase + Tg], nrm[:])
                    if l == 2:
                        og64 = out_g[:].rearrange("p (t d) -> p t d", d=64)
                        for i in range(3):
                            nc.vector.tensor_copy(
                                out=og64[:, :, 16 + i],
                                in_=inv_sb[i][:, gbase:gbase + Tg])
                    # update ego (bf16) and store piece (padded bf16)
                    nc.vector.tensor_copy(
                        out=ego_sb[:, gbase * 64:(gbase + Tg) * 64], in_=out_g[:])
                    staged = dpool.tile([128, Tg, 128], bf16, tag="staged")
                    nc.vector.memset(staged[:, :, 64:], 0.0)
                    nc.vector.tensor_copy(
                        out=staged[:, :, :64],
                        in_=out_g[:].rearrange("p (t d) -> p t d", d=64))
                    dstp = pieces[l][:].rearrange("(p t) d -> p (t d)", p=128)
                    nc.sync.dma_start(
                        dstp[:, gbase * 128:(gbase + Tg) * 128],
                        staged[:].rearrange("p t d -> p (t d)"))

                nc.gpsimd.collective_compute(
                    "AllGather", ALU.bypass,
                    replica_groups=[list(range(NCORES))],
                    ins=[pieces[l].opt()], outs=[tables[l + 1].opt()],
                )
                score_fetch(l + 1)

            # ================= scoring dots =================
            dls = [meta["D0"]] + douts
            acc = {}
            for which, o1 in (("pos", 128), ("neg", 256)):
                total = dpool.tile([128, B3], f32, tag=f"tot{which}")
                for ti in range(4):
                    own = stage[ti][0]
                    dl = dls[ti]
                    flat = own[:].rearrange("p c d -> p (c d)").rearrange(
                        "p (j x) -> p j x", x=384)
                    u = flat[:, :, 0:dl]
                    v = flat[:, :, o1:o1 + dl]
                    prod = dpool.tile([128, B3 * dl], f32, tag="prod")
                    nc.vector.tensor_tensor(out=prod[:], in0=u, in1=v, op=ALU.mult)
                    d = dpool.tile([128, B3], f32, tag=f"dot{ti}{which}")
                    nc.vector.reduce_sum(
                        out=d[:], in_=prod[:].rearrange("p (j d) -> p j d", d=dl),
                        axis=mybir.AxisListType.X)
                    acc[(ti, which)] = d
                own3 = stage[3][0]
                flat3 = own3[:].rearrange("p c d -> p (c d)").rearrange(
                    "p (j x) -> p j x", x=384)
                for ti in range(1, 4):
                    col = 16 + ti - 1
                    iu = flat3[:, :, col]
                    iv = flat3[:, :, o1 + col]
                    d = acc[(ti, which)]
                    nc.vector.tensor_tensor(out=d[:], in0=d[:], in1=iu, op=ALU.mult)
                    nc.vector.tensor_tensor(out=d[:], in0=d[:], in1=iv, op=ALU.mult)
                nc.vector.tensor_tensor(out=total[:], in0=acc[(0, which)][:],
                                        in1=acc[(1, which)][:], op=ALU.add)
                nc.vector.tensor_tensor(out=total[:], in0=total[:],
                                        in1=acc[(2, which)][:], op=ALU.add)
                nc.vector.tensor_tensor(out=total[:], in0=total[:],
                                        in1=acc[(3, which)][:], op=ALU.add)
                acc[which] = total
            outt = dpool.tile([128, 2 * B3], f32, tag="outt")
            nc.vector.tensor_copy(out=outt[:, :B3], in_=acc["pos"][:])
            nc.vector.tensor_copy(out=outt[:, B3:], in_=acc["neg"][:])
            nc.sync.dma_start(scores_out[:], outt[:])

    nc.compile()
    return nc


def kernel(**inputs):
    meta, in_maps = build_host_data(inputs)
    nc = build_program(meta)
    trace = os.environ.get("KGAT_TRACE", "0") == "1"
    rr = run_bass_kernel_spmd(nc, in_maps, list(range(NCORES)), trace=trace)
    if trace and rr.exec_time_ns is not None:
        print(f"HW exec time: {rr.exec_time_ns} ns")
    if trace and rr.profile_json is not None:
        import json
        with open("/tmp/kgat_profile.json", "w") as f:
            json.dump(rr.profile_json, f)
    res = rr.results
    B3, BPC, B = meta["B3"], meta["BPC"], meta["B"]
    out = np.zeros((B, 2), np.float32)
    for c in range(NCORES):
        sc = res[c]["scores"]                       # [128, 2*B3]
        pos = sc[:, :B3]                            # [128, B3] (partition, jj)
        neg = sc[:, B3:]
        b = np.arange(BPC)
        out[c * BPC + b, 0] = pos[b % 128, b // 128]
        out[c * BPC + b, 1] = neg[b % 128, b // 128]
    return out
